# revision 43
# baseline (speedup 1.0000x reference)
"""Llama-style GQA attention (B=1, S=2048, D=4096, 32 q-heads / 8 kv-heads,
rope, causal) on 8 trn2 NeuronCores, tensor-parallel over heads.

Core c owns q-heads 4c..4c+3 and kv-head c. Activations live in
"transposed" (feature-on-partition, seq-on-free) layout so every matmul
contracts over the partition dim. W_O is row-sharded; each core emits a
partial (D, S) bf16 output and the host sums the 8 partials and transposes.

v3: the scalar engine paces the attention phase (softmax exp at ~687ns per
(128,512) tile with a fixed ~260ns overhead each), so all PSUM tiles are
uniform 2-bank (128,1024) tiles: score pairs get exp'd with one ACTIVATE
(half the overhead count), projection accumulators pair two seq-chunks,
and ctx+den share one tile. The softmax denominator comes from a DVE bf16
tree-sum of the P tiles plus a single ones-column matmul per (head,
q-chunk) instead of a per-k-tile PE matmul (frees ~30us of PE), and its
normalize chain is software-pipelined one (h,j) iteration behind so the
PSUM ring never blocks on it. RoPE runs fully in bf16 (2x DVE rate). V is
transposed by the DMA XBAR instead of the PE. x streaming alternates two
DMA queues (one queue sustains only ~140GB/s, which starved the projection
phases), and output stores alternate the sync/scalar HWDGE queues.

RoPE trick: wq/wk rows are de-interleaved per head on the host
([0,2,..,126,1,3,..,127]) so the on-device pair (2j, 2j+1) becomes
(j, j+64) — a 64-partition block swap done with two partition-offset
vector ops against host-precomputed sign-folded cos/sin tables. The
permutation cancels in Q.K, and V/W_O are untouched by it.

Softmax is computed without max-subtraction (scores are bounded by
construction: |s| < ~10 => exp is safe in fp32), scoresT is held
(k on partitions, q on free) so P needs no transpose for P@V.
"""
import os
import numpy as np
import ml_dtypes

S = 2048
D = 4096
HD = 128
NCH = 4          # 512-wide seq chunks
KTILES = 32      # contraction tiles over D
CH = 512
SCALE = 1.0 / np.sqrt(128.0)

_cache = {}


def _build():
    import concourse.bacc as bacc
    import concourse.tile as tile
    import concourse.mybir as mybir
    from concourse import bass

    dt = mybir.dt
    nc = bacc.Bacc("TRN2", target_bir_lowering=False, debug=False,
                   enable_asserts=False, num_devices=8)

    def inp(name, shape, d):
        return nc.dram_tensor(name, shape, d, kind="ExternalInput").ap()

    xT = inp("xT", (D, S), dt.bfloat16)
    # packed weights: partition dim = contraction feature (128), free dim packs
    # the tile grid contiguously so one DMA covers many matmul lhsT tiles.
    wqp = inp("wqp", (HD, 16384), dt.bfloat16)   # col = g*8192 + k*256 + mi*128 + m
    wkp = inp("wkp", (HD, 4096), dt.bfloat16)    # col = k*128 + m
    wvp = inp("wvp", (HD, 4096), dt.bfloat16)
    wop = inp("wop", (HD, 16384), dt.bfloat16)   # col = of*512 + cf*128 + m
    cosd = inp("cosd", (HD, S), dt.bfloat16)
    sind = inp("sind", (HD, S), dt.bfloat16)
    maskd = inp("maskd", (HD, 4 * CH), dt.bfloat16)
    onesc = inp("onesc", (HD, 1), dt.bfloat16)
    outT = nc.dram_tensor("outT", (D, S), dt.bfloat16, kind="ExternalOutput").ap()

    f32 = dt.float32
    bf16 = dt.bfloat16
    Exp = mybir.ActivationFunctionType.Exp

    with tile.TileContext(nc) as tc:
        with (
            tc.tile_pool(name="const", bufs=1) as constp,
            tc.tile_pool(name="xs", bufs=11) as xpool,
            tc.tile_pool(name="wq", bufs=5) as wqpool,
            tc.tile_pool(name="wkv", bufs=2) as wkvpool,
            tc.tile_pool(name="wo", bufs=2) as wopool,
            tc.tile_pool(name="acts", bufs=1) as actp,
            tc.tile_pool(name="pt", bufs=5) as ptpool,
            tc.tile_pool(name="ds", bufs=4) as dspool,
            tc.tile_pool(name="tmp", bufs=2) as tmpp,
            tc.tile_pool(name="ost", bufs=2) as ostp,
            tc.tile_pool(name="ps", bufs=4, space="PSUM") as psp,
        ):
            # ---- resident x first half: gpsimd queue, issued first so k=0
            # arrives early; weights ride the sync HWDGE queue in parallel.
            xres = [actp.tile([HD, S], dt.bfloat16, tag=f"xres{k}", name=f"xres{k}")
                    for k in range(15)]
            # Startup fast path: the first wq chunk (split so the k=0 slice
            # lands first) rides scalar while xres[0] halves ride sync — the
            # first matmul's deps complete ~11us in instead of ~17us.
            # Remaining xres: evens on gpsimd, odds on scalar ahead of the
            # consts. One queue alone sustains only ~140GB/s — 8MB serial
            # would still be loading mid-phase and starve the k>=16 stream.
            wqc00 = wqpool.tile([HD, 1024], dt.bfloat16, tag="wq", name="wqc0_0")
            nc.scalar.dma_start(wqc00[:, 0:256], wqp[:, 0:256])
            nc.scalar.dma_start(wqc00[:, 256:1024], wqp[:, 256:1024])
            nc.sync.dma_start(xres[0][:, 0:2 * CH], xT[0:HD, 0:2 * CH])
            nc.sync.dma_start(xres[0][:, 2 * CH:S], xT[0:HD, 2 * CH:S])
            # wq chunks t=1..3 hoisted onto sync before the tail xres tiles so
            # the sync queue carries xres 12-15 (gpsimd/scalar alone deliver
            # the resident half with zero margin — any jitter became a PE gap)
            wq_pre = {0: wqc00}
            for t in (1, 2, 3):
                wt = wqpool.tile([HD, 1024], dt.bfloat16, tag="wq",
                                 name=f"wqc0_{t}")
                nc.sync.dma_start(wt[:], wqp[:, t * 1024:(t + 1) * 1024])
                wq_pre[t] = wt
            for k in range(1, 12):
                eng = nc.gpsimd if k % 2 == 0 else nc.scalar
                if k <= 5:   # halves: finer arrival granularity at cold start
                    eng.dma_start(xres[k][:, 0:2 * CH],
                                  xT[k * HD:(k + 1) * HD, 0:2 * CH])
                    eng.dma_start(xres[k][:, 2 * CH:S],
                                  xT[k * HD:(k + 1) * HD, 2 * CH:S])
                else:
                    eng.dma_start(xres[k][:], xT[k * HD:(k + 1) * HD, :])
            for k in range(12, 15):
                nc.sync.dma_start(xres[k][:], xT[k * HD:(k + 1) * HD, :])

            # ---- constants: tiles exist now, DMAs are emitted after the
            # Qg0 k-loop so they queue BEHIND its x stream (first use ~78us;
            # fronting them starved the k>=15 stream by ~4us)
            cos_t = constp.tile([HD, S], bf16, tag="cos")
            sin_t = constp.tile([HD, S], bf16, tag="sin")
            mask_t = constp.tile([HD, 4 * CH], bf16, tag="mask")
            onesc_t = constp.tile([HD, 1], bf16, tag="onesc")

            # persistent activations (bf16, feature x seq)
            qtr = [actp.tile([HD, S], bf16, tag=f"qtr{h}", name=f"qtr{h}") for h in range(4)]
            ktr = actp.tile([HD, S], bf16, tag="ktr")
            vbuf = actp.tile([HD, 16 * HD], bf16, tag="vbuf")  # (k 128, kt*128 d)
            ctxn = [actp.tile([HD, S], bf16, tag=f"ctx{h}", name=f"ctx{h}") for h in range(4)]

            def psum2(name):
                return psp.tile([HD, 2 * CH], f32, tag="mm2", name=name)

            def rope_into(dst, ps, ch):
                """dst[:, ch*512:...] (bf16) = st*COS + swap64(st)*SIN (all bf16)"""
                c0 = ch * CH
                st = tmpp.tile([HD, CH], bf16, tag="rst", bufs=6)
                nc.scalar.copy(st[:], ps)      # frees the PSUM half quickly
                t1 = tmpp.tile([HD, CH], bf16, tag="r1")
                nc.vector.tensor_mul(t1[:], st[:], cos_t[:, c0:c0 + CH])
                t2 = tmpp.tile([HD, CH], bf16, tag="r2")
                nc.vector.tensor_mul(t2[0:64, :], st[64:128, :], sin_t[64:128, c0:c0 + CH])
                nc.vector.tensor_mul(t2[64:128, :], st[0:64, :], sin_t[0:64, c0:c0 + CH])
                nc.vector.tensor_add(dst[:, c0:c0 + CH], t1[:], t2[:])

            # streamed x arrives as (128,1024) half-tiles cycling over all
            # three DMA queues — finer arrival granularity + deeper prefetch
            # than whole 512KB tiles on a ~140GB/s-per-queue fabric.
            xq = [0]

            def get_x(k):
                """Returns xsl(ch) -> (128,512) AP for seq-chunk ch of k-tile k."""
                if k < 15:
                    t = xres[k]
                    return lambda ch, t=t: t[:, ch * CH:(ch + 1) * CH]
                halves = []
                for u in range(2):
                    tt = xpool.tile([HD, 2 * CH], dt.bfloat16, tag="xt",
                                    name=f"xt{k}_{u}")
                    eng = [nc.sync, nc.gpsimd, nc.scalar][xq[0] % 3]
                    xq[0] += 1
                    eng.dma_start(tt[:], xT[k * HD:(k + 1) * HD,
                                            u * 2 * CH:(u + 1) * 2 * CH])
                    halves.append(tt)
                return lambda ch, hs=halves: hs[ch // 2][:, (ch % 2) * CH:
                                                          (ch % 2 + 1) * CH]

            # ---- Q projection: two groups of 2 head-tiles ----
            # accumulators pair seq-chunks: qps[mi][cp] covers ch = 2cp, 2cp+1
            for g in range(2):
                qps = [[psum2(f"qps{g}_{mi}_{cp}") for cp in range(2)]
                       for mi in range(2)]
                if g == 0:
                    # HAM warmup: ~3.5us of dependency-free matmuls during the
                    # startup DMA window so the PE clock-gate opens (1.2 ->
                    # 2.4GHz) before the first real matmul. Output lands in a
                    # qps slot that the real k=0 matmul clears via start=True.
                    scr = constp.tile([HD, 64], bf16, tag="scr")
                    nc.vector.memset(scr[:], 0)
                    for w in range(56):
                        nc.tensor.matmul(qps[0][0][0:64, 0:64], scr[:], scr[:],
                                         start=True, stop=True)
                for t in range(8):   # wq chunk = 4 k-tiles, one 256KB DMA
                    if g == 0 and t in wq_pre:
                        wqc = wq_pre[t]
                    else:
                        wqc = wqpool.tile([HD, 1024], bf16, tag="wq",
                                          name=f"wqc{g}_{t}")
                        nc.sync.dma_start(wqc[:], wqp[:, g * 8192 + t * 1024:
                                                     g * 8192 + (t + 1) * 1024])
                    for dk in range(4):
                        k = 4 * t + dk
                        xsl = get_x(k)
                        for mi in range(2):
                            lhs = wqc[:, dk * 256 + mi * HD:dk * 256 + (mi + 1) * HD]
                            for ch in range(NCH):
                                nc.tensor.matmul(
                                    qps[mi][ch // 2][:, (ch % 2) * CH:(ch % 2 + 1) * CH],
                                    lhs, xsl(ch),
                                    start=(k == 0), stop=(k == KTILES - 1))
                if g == 0:
                    nc.sync.dma_start(cos_t[:], cosd[:])
                    nc.sync.dma_start(sin_t[:], sind[:])
                    nc.scalar.dma_start(mask_t[:], maskd[:])
                    nc.scalar.dma_start(onesc_t[:], onesc[:])
                for mi in range(2):
                    for ch in range(NCH):
                        rope_into(qtr[g * 2 + mi],
                                  qps[mi][ch // 2][:, (ch % 2) * CH:(ch % 2 + 1) * CH],
                                  ch)

            # ---- K + V projections ----
            kps = [psum2(f"kps{cp}") for cp in range(2)]
            vps = [psum2(f"vps{cp}") for cp in range(2)]
            # streamed k-tiles first, resident last: the tail of the KV loop
            # then has zero DMA dependence, so the KV->attn transition drains
            # immediately instead of stalling on just-in-time stream arrivals
            kv_order = list(range(15, KTILES)) + list(range(15))
            for t in range(4):   # wk/wv chunk = 8 k-tiles, one 256KB DMA each
                wkc = wkvpool.tile([HD, 1024], bf16, tag="wk", name=f"wkc{t}")
                nc.sync.dma_start(wkc[:], wkp[:, t * 1024:(t + 1) * 1024])
                wvc = wkvpool.tile([HD, 1024], bf16, tag="wv", name=f"wvc{t}")
                nc.sync.dma_start(wvc[:], wvp[:, t * 1024:(t + 1) * 1024])
                for dk in range(8):
                    kk = 8 * t + dk
                    k = kv_order[kk]
                    xsl = get_x(k)
                    for ch in range(NCH):
                        nc.tensor.matmul(
                            kps[ch // 2][:, (ch % 2) * CH:(ch % 2 + 1) * CH],
                            wkc[:, dk * HD:(dk + 1) * HD], xsl(ch),
                            start=(kk == 0), stop=(kk == KTILES - 1))
                        nc.tensor.matmul(
                            vps[ch // 2][:, (ch % 2) * CH:(ch % 2 + 1) * CH],
                            wvc[:, dk * HD:(dk + 1) * HD], xsl(ch),
                            start=(kk == 0), stop=(kk == KTILES - 1))
            for ch in range(NCH):
                rope_into(ktr, kps[ch // 2][:, (ch % 2) * CH:(ch % 2 + 1) * CH], ch)
            # V: stage bf16, then DMA-XBAR block transpose into (seq, d) layout
            vstage = actp.tile([HD, S], bf16, tag="vstage")
            for ch in range(NCH):
                nc.vector.tensor_copy(
                    vstage[:, ch * CH:(ch + 1) * CH],
                    vps[ch // 2][:, (ch % 2) * CH:(ch % 2 + 1) * CH])
                for st in range(4 * ch, 4 * ch + 4):
                    nc.sync.dma_start_transpose(
                        vbuf[:, st * HD:(st + 1) * HD],
                        vstage[:, st * HD:(st + 1) * HD])

            # ---- attention, per head / q-chunk (paired k-tiles) ----
            # finalize (den matmul + reciprocal + broadcast + normalize) is
            # pipelined one (h,j) behind so its latency hides under the next
            # block's matmuls and the PSUM ring never waits on it.
            def finalize(fin):
                cd, h, j = fin
                recip = tmpp.tile([1, CH], f32, tag="recip")
                nc.vector.reciprocal_approx_fast(recip[:], cd[0:1, CH:2 * CH])
                bcs = tmpp.tile([HD, CH], f32, tag="bcs")
                nc.gpsimd.partition_broadcast(bcs[:], recip[:], channels=HD)
                nc.vector.tensor_mul(ctxn[h][:, j * CH:(j + 1) * CH],
                                     cd[:, 0:CH], bcs[:])

            pending = None
            for h in range(4):
                for j in range(NCH):
                    q0 = j * CH
                    ktmax = 4 * (j + 1)
                    npair = ktmax // 2
                    ctxden = psum2(f"cd{h}_{j}")   # ctx in half0, den row in half1
                    pts = []     # one (128,1024) bf16 tile per k-tile pair
                    def score_exp_pair(pr):
                        sps = psum2(f"sps{h}_{j}_{pr}")
                        for u in range(2):
                            kt = 2 * pr + u
                            nc.tensor.matmul(sps[:, u * CH:(u + 1) * CH],
                                             ktr[:, kt * HD:(kt + 1) * HD],
                                             qtr[h][:, q0:q0 + CH],
                                             start=True, stop=True)
                        pt = ptpool.tile([HD, 2 * CH], bf16, tag="pt",
                                         name=f"pt{h}_{j}_{pr}")
                        nc.scalar.activation(pt[:], sps[:], Exp, scale=SCALE)
                        if pr >= 2 * j:          # both k-tiles in the diagonal blocks
                            mp = pr - 2 * j
                            ptm = ptpool.tile([HD, 2 * CH], bf16, tag="pt",
                                              name=f"ptm{h}_{j}_{pr}")
                            nc.vector.tensor_mul(
                                ptm[:], pt[:], mask_t[:, mp * 2 * CH:(mp + 1) * 2 * CH])
                            pt = ptm
                        pts.append(pt)
                    def pv_pair(pr):   # lag-issued so PE overlaps ACT exp
                        for u in range(2):
                            kt = 2 * pr + u
                            nc.tensor.matmul(ctxden[:, 0:CH],
                                             vbuf[:, kt * HD:(kt + 1) * HD],
                                             pts[pr][:, u * CH:(u + 1) * CH],
                                             start=(kt == 0), stop=(kt == ktmax - 1))
                        # den: two P pairs summed elementwise (one 1024-wide
                        # DVE add), halves folded (512-wide add), then a
                        # ones-column matmul accumulates into ctxden's 2nd bank
                        if pr % 2 == 1:
                            gq = pr // 2
                            pa = dspool.tile([HD, 2 * CH], bf16, tag="da",
                                             bufs=2, name=f"da{h}_{j}_{gq}")
                            nc.vector.tensor_add(pa[:], pts[pr - 1][:],
                                                 pts[pr][:])
                            hs = dspool.tile([HD, CH], bf16, tag="dc", bufs=2,
                                             name=f"dh{h}_{j}_{gq}")
                            nc.vector.tensor_add(hs[:], pa[:, 0:CH],
                                                 pa[:, CH:2 * CH])
                            nc.tensor.matmul(ctxden[0:1, CH:2 * CH], onesc_t[:],
                                             hs[:], start=(gq == 0),
                                             stop=(gq == npair // 2 - 1))
                    if pending is not None:
                        finalize(pending)
                        pending = None
                    PLAG = 1
                    for pr in range(npair + PLAG):
                        if pr < npair:
                            score_exp_pair(pr)
                        if pr >= PLAG:
                            pv_pair(pr - PLAG)
                    pending = (ctxden, h, j)
            finalize(pending)

            # ---- O projection (row-sharded W_O -> partial outT, bf16) ----
            for t in range(8):   # wo chunk = 4 of-tiles, one 512KB DMA
                woc = wopool.tile([HD, 2048], bf16, tag="wo", name=f"woc{t}")
                nc.scalar.dma_start(woc[:], wop[:, t * 2048:(t + 1) * 2048])
                for oo in range(4):
                    of = 4 * t + oo
                    ops = [psum2(f"ops{of}_{cp}") for cp in range(2)]
                    for cf in range(4):
                        lhs = woc[:, oo * 512 + cf * HD:oo * 512 + (cf + 1) * HD]
                        for ch in range(NCH):
                            nc.tensor.matmul(
                                ops[ch // 2][:, (ch % 2) * CH:(ch % 2 + 1) * CH],
                                lhs, ctxn[cf][:, ch * CH:(ch + 1) * CH],
                                start=(cf == 0), stop=(cf == 3))
                    ost = ostp.tile([HD, S], bf16, tag="ost", name=f"ost{of}")
                    for ch in range(NCH):
                        src = ops[ch // 2][:, (ch % 2) * CH:(ch % 2 + 1) * CH]
                        dst = ost[:, ch * CH:(ch + 1) * CH]
                        if of == 31 and ch % 2 == 1:
                            nc.scalar.copy(dst, src)
                        else:
                            nc.vector.tensor_copy(dst, src)
                    if of >= 30:   # drain the tail on both queues, per chunk
                        for ch in range(NCH):
                            oeng = nc.sync if ch % 2 == 0 else nc.scalar
                            oeng.dma_start(
                                outT[of * HD:(of + 1) * HD,
                                     ch * CH:(ch + 1) * CH],
                                ost[:, ch * CH:(ch + 1) * CH])
                    else:
                        oeng = nc.sync if of % 2 == 0 else nc.scalar
                        oeng.dma_start(outT[of * HD:(of + 1) * HD, :], ost[:])

    nc.compile()
    return nc


def _host_inputs(x, wq, wk, wv, wo):
    bf16 = ml_dtypes.bfloat16
    perm = np.concatenate([np.arange(0, 128, 2), np.arange(1, 128, 2)])
    half = 64
    inv = 1.0 / (10000.0 ** (np.arange(half) / half))
    ang = np.arange(S)[:, None] * inv[None, :]
    cosd = np.ascontiguousarray(
        np.concatenate([np.cos(ang).T, np.cos(ang).T], 0)).astype(bf16)
    sind = np.ascontiguousarray(
        np.concatenate([np.sin(ang).T, -np.sin(ang).T], 0)).astype(bf16)
    maskd = np.zeros((HD, 4 * CH), np.float32)
    for m in range(4):
        kl = np.arange(HD)[:, None]
        maskd[:, m * CH:(m + 1) * CH] = (np.arange(CH)[None, :] >= HD * m + kl)
    maskd = maskd.astype(bf16)
    onescol = np.ones((HD, 1), bf16)
    xTb = np.ascontiguousarray(x[0].T).astype(bf16)

    in_maps = []
    for c in range(8):
        qrows = slice(512 * c, 512 * (c + 1))
        wq_c = wq[qrows].reshape(4, HD, D)[:, perm].reshape(512, D)
        # wqp[p, g*8192 + k*256 + mi*128 + m] = wq_c[g*256+mi*128+m, k*128+p]
        wqp = np.ascontiguousarray(
            wq_c.reshape(2, 2, HD, KTILES, HD).transpose(4, 0, 3, 1, 2)
            .reshape(HD, 16384)).astype(bf16)
        kv_order = np.array(list(range(15, KTILES)) + list(range(15)))
        wk_c = wk[HD * c:HD * (c + 1)][perm]
        # wkp[p, t*128 + m] = wk_c[m, kv_order[t]*128+p]
        wkp = np.ascontiguousarray(
            wk_c.reshape(HD, KTILES, HD)[:, kv_order].transpose(2, 1, 0)
            .reshape(HD, 4096)).astype(bf16)
        wv_c = wv[HD * c:HD * (c + 1)]
        wvp = np.ascontiguousarray(
            wv_c.reshape(HD, KTILES, HD)[:, kv_order].transpose(2, 1, 0)
            .reshape(HD, 4096)).astype(bf16)
        woT_c = wo[:, qrows].T            # (512 ctx-feat, 4096 out-feat)
        # wop[p, of*512 + cf*128 + m] = woT_c[cf*128+p, of*128+m]
        wop = np.ascontiguousarray(
            woT_c.reshape(4, HD, KTILES, HD).transpose(1, 2, 0, 3)
            .reshape(HD, 16384)).astype(bf16)
        in_maps.append({
            "xT": xTb, "wqp": wqp, "wkp": wkp, "wvp": wvp, "wop": wop,
            "cosd": cosd, "sind": sind, "maskd": maskd, "onesc": onescol,
        })
    return in_maps


LAST_RESULTS = None


def kernel(x, wq, wk, wv, wo, attn_mask):
    global LAST_RESULTS
    from concourse import bass_utils
    if "nc" not in _cache:
        _cache["nc"] = _build()
    nc = _cache["nc"]
    in_maps = _host_inputs(np.asarray(x, np.float32), np.asarray(wq, np.float32),
                           np.asarray(wk, np.float32), np.asarray(wv, np.float32),
                           np.asarray(wo, np.float32))
    res = bass_utils.run_bass_kernel_spmd(
        nc, in_maps, core_ids=list(range(8)),
        trace=bool(os.environ.get("BASS_TRACE")))
    LAST_RESULTS = res
    acc = res.results[0]["outT"].astype(np.float64)
    for c in range(1, 8):
        acc = acc + res.results[c]["outT"].astype(np.float64)
    return np.ascontiguousarray(acc.T).astype(np.float32).reshape(1, S, D)


# revision 44
# speedup vs baseline: 1.0434x; 1.0434x over previous
"""Llama-style GQA attention (B=1, S=2048, D=4096, 32 q-heads / 8 kv-heads,
rope, causal) on 8 trn2 NeuronCores, tensor-parallel over heads.

Core c owns q-heads 4c..4c+3 and kv-head c. Activations live in
"transposed" (feature-on-partition, seq-on-free) layout so every matmul
contracts over the partition dim. W_O is row-sharded; each core emits a
partial (D, S) bf16 output and the host sums the 8 partials and transposes.

v3: the scalar engine paces the attention phase (softmax exp at ~687ns per
(128,512) tile with a fixed ~260ns overhead each), so all PSUM tiles are
uniform 2-bank (128,1024) tiles: score pairs get exp'd with one ACTIVATE
(half the overhead count), projection accumulators pair two seq-chunks,
and ctx+den share one tile. The softmax denominator comes from a DVE bf16
tree-sum of the P tiles plus a single ones-column matmul per (head,
q-chunk) instead of a per-k-tile PE matmul (frees ~30us of PE), and its
normalize chain is software-pipelined one (h,j) iteration behind so the
PSUM ring never blocks on it. RoPE runs fully in bf16 (2x DVE rate). V is
transposed by the DMA XBAR instead of the PE. x streaming alternates two
DMA queues (one queue sustains only ~140GB/s, which starved the projection
phases), and output stores alternate the sync/scalar HWDGE queues.

RoPE trick: wq/wk rows are de-interleaved per head on the host
([0,2,..,126,1,3,..,127]) so the on-device pair (2j, 2j+1) becomes
(j, j+64) — a 64-partition block swap done with two partition-offset
vector ops against host-precomputed sign-folded cos/sin tables. The
permutation cancels in Q.K, and V/W_O are untouched by it.

Softmax is computed without max-subtraction (scores are bounded by
construction: |s| < ~10 => exp is safe in fp32), scoresT is held
(k on partitions, q on free) so P needs no transpose for P@V.
"""
import os
import numpy as np
import ml_dtypes

S = 2048
D = 4096
HD = 128
NCH = 4          # 512-wide seq chunks
KTILES = 32      # contraction tiles over D
CH = 512
SCALE = 1.0 / np.sqrt(128.0)

_cache = {}


def _build():
    import concourse.bacc as bacc
    import concourse.tile as tile
    import concourse.mybir as mybir
    from concourse import bass

    dt = mybir.dt
    nc = bacc.Bacc("TRN2", target_bir_lowering=False, debug=False,
                   enable_asserts=False, num_devices=8)

    def inp(name, shape, d):
        return nc.dram_tensor(name, shape, d, kind="ExternalInput").ap()

    xT = inp("xT", (D, S), dt.bfloat16)
    # packed weights: partition dim = contraction feature (128), free dim packs
    # the tile grid contiguously so one DMA covers many matmul lhsT tiles.
    wqp = inp("wqp", (HD, 16384), dt.bfloat16)   # col = g*8192 + k*256 + mi*128 + m
    wkp = inp("wkp", (HD, 4096), dt.bfloat16)    # col = k*128 + m
    wvp = inp("wvp", (HD, 4096), dt.bfloat16)
    wop = inp("wop", (HD, 16384), dt.bfloat16)   # col = of*512 + cf*128 + m
    cosd = inp("cosd", (HD, S), dt.bfloat16)
    sind = inp("sind", (HD, S), dt.bfloat16)
    maskd = inp("maskd", (HD, 4 * CH), dt.bfloat16)
    onesc = inp("onesc", (HD, 1), dt.bfloat16)
    outT = nc.dram_tensor("outT", (D, S), dt.bfloat16, kind="ExternalOutput").ap()

    f32 = dt.float32
    bf16 = dt.bfloat16
    Exp = mybir.ActivationFunctionType.Exp

    with tile.TileContext(nc) as tc:
        with (
            tc.tile_pool(name="const", bufs=1) as constp,
            tc.tile_pool(name="xs", bufs=11) as xpool,
            tc.tile_pool(name="wq", bufs=5) as wqpool,
            tc.tile_pool(name="wkv", bufs=2) as wkvpool,
            tc.tile_pool(name="wo", bufs=2) as wopool,
            tc.tile_pool(name="acts", bufs=1) as actp,
            tc.tile_pool(name="pt", bufs=5) as ptpool,
            tc.tile_pool(name="ds", bufs=4) as dspool,
            tc.tile_pool(name="tmp", bufs=2) as tmpp,
            tc.tile_pool(name="ost", bufs=2) as ostp,
            tc.tile_pool(name="ps", bufs=4, space="PSUM") as psp,
        ):
            # ---- resident x first half: gpsimd queue, issued first so k=0
            # arrives early; weights ride the sync HWDGE queue in parallel.
            xres = [actp.tile([HD, S], dt.bfloat16, tag=f"xres{k}", name=f"xres{k}")
                    for k in range(15)]
            # Startup fast path: the first wq chunk (split so the k=0 slice
            # lands first) rides scalar while xres[0] halves ride sync — the
            # first matmul's deps complete ~11us in instead of ~17us.
            # Remaining xres: evens on gpsimd, odds on scalar ahead of the
            # consts. One queue alone sustains only ~140GB/s — 8MB serial
            # would still be loading mid-phase and starve the k>=16 stream.
            wqc00 = wqpool.tile([HD, 1024], dt.bfloat16, tag="wq", name="wqc0_0")
            nc.scalar.dma_start(wqc00[:, 0:256], wqp[:, 0:256])
            nc.scalar.dma_start(wqc00[:, 256:1024], wqp[:, 256:1024])
            nc.sync.dma_start(xres[0][:, 0:2 * CH], xT[0:HD, 0:2 * CH])
            nc.sync.dma_start(xres[0][:, 2 * CH:S], xT[0:HD, 2 * CH:S])
            # wq chunks t=1..3 hoisted onto sync before the tail xres tiles so
            # the sync queue carries xres 12-15 (gpsimd/scalar alone deliver
            # the resident half with zero margin — any jitter became a PE gap)
            wq_pre = {0: wqc00}
            for t in (1, 2, 3):
                wt = wqpool.tile([HD, 1024], dt.bfloat16, tag="wq",
                                 name=f"wqc0_{t}")
                nc.sync.dma_start(wt[:], wqp[:, t * 1024:(t + 1) * 1024])
                wq_pre[t] = wt
            for k in range(1, 12):
                eng = nc.gpsimd if k % 2 == 0 else nc.scalar
                if k <= 5:   # halves: finer arrival granularity at cold start
                    eng.dma_start(xres[k][:, 0:2 * CH],
                                  xT[k * HD:(k + 1) * HD, 0:2 * CH])
                    eng.dma_start(xres[k][:, 2 * CH:S],
                                  xT[k * HD:(k + 1) * HD, 2 * CH:S])
                else:
                    eng.dma_start(xres[k][:], xT[k * HD:(k + 1) * HD, :])
            for k in range(12, 15):
                nc.sync.dma_start(xres[k][:], xT[k * HD:(k + 1) * HD, :])

            # ---- constants: tiles exist now, DMAs are emitted after the
            # Qg0 k-loop so they queue BEHIND its x stream (first use ~78us;
            # fronting them starved the k>=15 stream by ~4us)
            cos_t = constp.tile([HD, S], bf16, tag="cos")
            sin_t = constp.tile([HD, S], bf16, tag="sin")
            mask_t = constp.tile([HD, 4 * CH], bf16, tag="mask")
            onesc_t = constp.tile([HD, 1], bf16, tag="onesc")

            # persistent activations (bf16, feature x seq)
            qtr = [actp.tile([HD, S], bf16, tag=f"qtr{h}", name=f"qtr{h}") for h in range(4)]
            ktr = actp.tile([HD, S], bf16, tag="ktr")
            vbuf = actp.tile([HD, 16 * HD], bf16, tag="vbuf")  # (k 128, kt*128 d)
            ctxn = [actp.tile([HD, S], bf16, tag=f"ctx{h}", name=f"ctx{h}") for h in range(4)]

            def psum2(name):
                return psp.tile([HD, 2 * CH], f32, tag="mm2", name=name)

            def rope_into(dst, ps, ch):
                """dst[:, ch*512:...] (bf16) = st*COS + swap64(st)*SIN (all bf16)"""
                c0 = ch * CH
                st = tmpp.tile([HD, CH], bf16, tag="rst", bufs=6)
                nc.scalar.copy(st[:], ps)      # frees the PSUM half quickly
                t1 = tmpp.tile([HD, CH], bf16, tag="r1")
                nc.vector.tensor_mul(t1[:], st[:], cos_t[:, c0:c0 + CH])
                t2 = tmpp.tile([HD, CH], bf16, tag="r2")
                nc.vector.tensor_mul(t2[0:64, :], st[64:128, :], sin_t[64:128, c0:c0 + CH])
                nc.vector.tensor_mul(t2[64:128, :], st[0:64, :], sin_t[0:64, c0:c0 + CH])
                nc.vector.tensor_add(dst[:, c0:c0 + CH], t1[:], t2[:])

            # streamed x arrives as (128,1024) half-tiles cycling over all
            # three DMA queues — finer arrival granularity + deeper prefetch
            # than whole 512KB tiles on a ~140GB/s-per-queue fabric.
            xq = [0]

            def get_x(k):
                """Returns xsl(ch) -> (128,512) AP for seq-chunk ch of k-tile k."""
                if k < 15:
                    t = xres[k]
                    return lambda ch, t=t: t[:, ch * CH:(ch + 1) * CH]
                halves = []
                for u in range(2):
                    tt = xpool.tile([HD, 2 * CH], dt.bfloat16, tag="xt",
                                    name=f"xt{k}_{u}")
                    eng = [nc.sync, nc.gpsimd, nc.scalar][xq[0] % 3]
                    xq[0] += 1
                    eng.dma_start(tt[:], xT[k * HD:(k + 1) * HD,
                                            u * 2 * CH:(u + 1) * 2 * CH])
                    halves.append(tt)
                return lambda ch, hs=halves: hs[ch // 2][:, (ch % 2) * CH:
                                                          (ch % 2 + 1) * CH]

            # ---- Q projection: two groups of 2 head-tiles ----
            # accumulators pair seq-chunks: qps[mi][cp] covers ch = 2cp, 2cp+1
            for g in range(2):
                qps = [[psum2(f"qps{g}_{mi}_{cp}") for cp in range(2)]
                       for mi in range(2)]
                if g == 0:
                    # HAM warmup: ~3.5us of dependency-free matmuls during the
                    # startup DMA window so the PE clock-gate opens (1.2 ->
                    # 2.4GHz) before the first real matmul. Output lands in a
                    # qps slot that the real k=0 matmul clears via start=True.
                    scr = constp.tile([HD, 64], bf16, tag="scr")
                    nc.vector.memset(scr[:], 0)
                    for w in range(56):
                        nc.tensor.matmul(qps[0][0][0:64, 0:64], scr[:], scr[:],
                                         start=True, stop=True)
                for t in range(8):   # wq chunk = 4 k-tiles, one 256KB DMA
                    if g == 0 and t in wq_pre:
                        wqc = wq_pre[t]
                    else:
                        wqc = wqpool.tile([HD, 1024], bf16, tag="wq",
                                          name=f"wqc{g}_{t}")
                        nc.sync.dma_start(wqc[:], wqp[:, g * 8192 + t * 1024:
                                                     g * 8192 + (t + 1) * 1024])
                    for dk in range(4):
                        k = 4 * t + dk
                        xsl = get_x(k)
                        for mi in range(2):
                            lhs = wqc[:, dk * 256 + mi * HD:dk * 256 + (mi + 1) * HD]
                            for ch in range(NCH):
                                nc.tensor.matmul(
                                    qps[mi][ch // 2][:, (ch % 2) * CH:(ch % 2 + 1) * CH],
                                    lhs, xsl(ch),
                                    start=(k == 0), stop=(k == KTILES - 1))
                if g == 0:
                    nc.sync.dma_start(cos_t[:], cosd[:])
                    nc.sync.dma_start(sin_t[:], sind[:])
                    nc.scalar.dma_start(mask_t[:], maskd[:])
                    nc.scalar.dma_start(onesc_t[:], onesc[:])
                for mi in range(2):
                    for ch in range(NCH):
                        rope_into(qtr[g * 2 + mi],
                                  qps[mi][ch // 2][:, (ch % 2) * CH:(ch % 2 + 1) * CH],
                                  ch)

            # ---- K + V projections ----
            kps = [psum2(f"kps{cp}") for cp in range(2)]
            vps = [psum2(f"vps{cp}") for cp in range(2)]
            for t in range(4):   # wk/wv chunk = 8 k-tiles, one 256KB DMA each
                wkc = wkvpool.tile([HD, 1024], bf16, tag="wk", name=f"wkc{t}")
                nc.sync.dma_start(wkc[:], wkp[:, t * 1024:(t + 1) * 1024])
                wvc = wkvpool.tile([HD, 1024], bf16, tag="wv", name=f"wvc{t}")
                nc.sync.dma_start(wvc[:], wvp[:, t * 1024:(t + 1) * 1024])
                for dk in range(8):
                    k = 8 * t + dk
                    xsl = get_x(k)
                    for ch in range(NCH):
                        nc.tensor.matmul(
                            kps[ch // 2][:, (ch % 2) * CH:(ch % 2 + 1) * CH],
                            wkc[:, dk * HD:(dk + 1) * HD], xsl(ch),
                            start=(k == 0), stop=(k == KTILES - 1))
                        nc.tensor.matmul(
                            vps[ch // 2][:, (ch % 2) * CH:(ch % 2 + 1) * CH],
                            wvc[:, dk * HD:(dk + 1) * HD], xsl(ch),
                            start=(k == 0), stop=(k == KTILES - 1))
            for ch in range(NCH):
                rope_into(ktr, kps[ch // 2][:, (ch % 2) * CH:(ch % 2 + 1) * CH], ch)
            # V: stage bf16, then DMA-XBAR block transpose into (seq, d) layout
            vstage = actp.tile([HD, S], bf16, tag="vstage")
            for ch in range(NCH):
                nc.vector.tensor_copy(
                    vstage[:, ch * CH:(ch + 1) * CH],
                    vps[ch // 2][:, (ch % 2) * CH:(ch % 2 + 1) * CH])
                for st in range(4 * ch, 4 * ch + 4):
                    nc.sync.dma_start_transpose(
                        vbuf[:, st * HD:(st + 1) * HD],
                        vstage[:, st * HD:(st + 1) * HD])

            # ---- attention, per head / q-chunk (paired k-tiles) ----
            # finalize (den matmul + reciprocal + broadcast + normalize) is
            # pipelined one (h,j) behind so its latency hides under the next
            # block's matmuls and the PSUM ring never waits on it.
            def finalize(fin):
                cd, h, j = fin
                recip = tmpp.tile([1, CH], f32, tag="recip")
                nc.vector.reciprocal_approx_fast(recip[:], cd[0:1, CH:2 * CH])
                bcs = tmpp.tile([HD, CH], f32, tag="bcs")
                nc.gpsimd.partition_broadcast(bcs[:], recip[:], channels=HD)
                nc.vector.tensor_mul(ctxn[h][:, j * CH:(j + 1) * CH],
                                     cd[:, 0:CH], bcs[:])

            pending = None
            for h in range(4):
                for j in range(NCH):
                    q0 = j * CH
                    ktmax = 4 * (j + 1)
                    npair = ktmax // 2
                    ctxden = psum2(f"cd{h}_{j}")   # ctx in half0, den row in half1
                    pts = []     # one (128,1024) bf16 tile per k-tile pair
                    def score_exp_pair(pr):
                        sps = psum2(f"sps{h}_{j}_{pr}")
                        for u in range(2):
                            kt = 2 * pr + u
                            nc.tensor.matmul(sps[:, u * CH:(u + 1) * CH],
                                             ktr[:, kt * HD:(kt + 1) * HD],
                                             qtr[h][:, q0:q0 + CH],
                                             start=True, stop=True)
                        pt = ptpool.tile([HD, 2 * CH], bf16, tag="pt",
                                         name=f"pt{h}_{j}_{pr}")
                        nc.scalar.activation(pt[:], sps[:], Exp, scale=SCALE)
                        if pr >= 2 * j:          # both k-tiles in the diagonal blocks
                            mp = pr - 2 * j
                            ptm = ptpool.tile([HD, 2 * CH], bf16, tag="pt",
                                              name=f"ptm{h}_{j}_{pr}")
                            nc.vector.tensor_mul(
                                ptm[:], pt[:], mask_t[:, mp * 2 * CH:(mp + 1) * 2 * CH])
                            pt = ptm
                        pts.append(pt)
                    def pv_pair(pr):   # lag-issued so PE overlaps ACT exp
                        for u in range(2):
                            kt = 2 * pr + u
                            nc.tensor.matmul(ctxden[:, 0:CH],
                                             vbuf[:, kt * HD:(kt + 1) * HD],
                                             pts[pr][:, u * CH:(u + 1) * CH],
                                             start=(kt == 0), stop=(kt == ktmax - 1))
                        # den: two P pairs summed elementwise (one 1024-wide
                        # DVE add), halves folded (512-wide add), then a
                        # ones-column matmul accumulates into ctxden's 2nd bank
                        if pr % 2 == 1:
                            gq = pr // 2
                            pa = dspool.tile([HD, 2 * CH], bf16, tag="da",
                                             bufs=2, name=f"da{h}_{j}_{gq}")
                            nc.vector.tensor_add(pa[:], pts[pr - 1][:],
                                                 pts[pr][:])
                            hs = dspool.tile([HD, CH], bf16, tag="dc", bufs=2,
                                             name=f"dh{h}_{j}_{gq}")
                            nc.vector.tensor_add(hs[:], pa[:, 0:CH],
                                                 pa[:, CH:2 * CH])
                            nc.tensor.matmul(ctxden[0:1, CH:2 * CH], onesc_t[:],
                                             hs[:], start=(gq == 0),
                                             stop=(gq == npair // 2 - 1))
                    if pending is not None:
                        finalize(pending)
                        pending = None
                    PLAG = 1
                    for pr in range(npair + PLAG):
                        if pr < npair:
                            score_exp_pair(pr)
                        if pr >= PLAG:
                            pv_pair(pr - PLAG)
                    pending = (ctxden, h, j)
            finalize(pending)

            # ---- O projection (row-sharded W_O -> partial outT, bf16) ----
            for t in range(8):   # wo chunk = 4 of-tiles, one 512KB DMA
                woc = wopool.tile([HD, 2048], bf16, tag="wo", name=f"woc{t}")
                nc.scalar.dma_start(woc[:], wop[:, t * 2048:(t + 1) * 2048])
                for oo in range(4):
                    of = 4 * t + oo
                    ops = [psum2(f"ops{of}_{cp}") for cp in range(2)]
                    for cf in range(4):
                        lhs = woc[:, oo * 512 + cf * HD:oo * 512 + (cf + 1) * HD]
                        for ch in range(NCH):
                            nc.tensor.matmul(
                                ops[ch // 2][:, (ch % 2) * CH:(ch % 2 + 1) * CH],
                                lhs, ctxn[cf][:, ch * CH:(ch + 1) * CH],
                                start=(cf == 0), stop=(cf == 3))
                    ost = ostp.tile([HD, S], bf16, tag="ost", name=f"ost{of}")
                    for ch in range(NCH):
                        src = ops[ch // 2][:, (ch % 2) * CH:(ch % 2 + 1) * CH]
                        dst = ost[:, ch * CH:(ch + 1) * CH]
                        if of == 31 and ch % 2 == 1:
                            nc.scalar.copy(dst, src)
                        else:
                            nc.vector.tensor_copy(dst, src)
                    if of >= 30:   # drain the tail on both queues, per chunk
                        for ch in range(NCH):
                            oeng = nc.sync if ch % 2 == 0 else nc.scalar
                            oeng.dma_start(
                                outT[of * HD:(of + 1) * HD,
                                     ch * CH:(ch + 1) * CH],
                                ost[:, ch * CH:(ch + 1) * CH])
                    else:
                        oeng = nc.sync if of % 2 == 0 else nc.scalar
                        oeng.dma_start(outT[of * HD:(of + 1) * HD, :], ost[:])

    nc.compile()
    return nc


def _host_inputs(x, wq, wk, wv, wo):
    bf16 = ml_dtypes.bfloat16
    perm = np.concatenate([np.arange(0, 128, 2), np.arange(1, 128, 2)])
    half = 64
    inv = 1.0 / (10000.0 ** (np.arange(half) / half))
    ang = np.arange(S)[:, None] * inv[None, :]
    cosd = np.ascontiguousarray(
        np.concatenate([np.cos(ang).T, np.cos(ang).T], 0)).astype(bf16)
    sind = np.ascontiguousarray(
        np.concatenate([np.sin(ang).T, -np.sin(ang).T], 0)).astype(bf16)
    maskd = np.zeros((HD, 4 * CH), np.float32)
    for m in range(4):
        kl = np.arange(HD)[:, None]
        maskd[:, m * CH:(m + 1) * CH] = (np.arange(CH)[None, :] >= HD * m + kl)
    maskd = maskd.astype(bf16)
    onescol = np.ones((HD, 1), bf16)
    xTb = np.ascontiguousarray(x[0].T).astype(bf16)

    in_maps = []
    for c in range(8):
        qrows = slice(512 * c, 512 * (c + 1))
        wq_c = wq[qrows].reshape(4, HD, D)[:, perm].reshape(512, D)
        # wqp[p, g*8192 + k*256 + mi*128 + m] = wq_c[g*256+mi*128+m, k*128+p]
        wqp = np.ascontiguousarray(
            wq_c.reshape(2, 2, HD, KTILES, HD).transpose(4, 0, 3, 1, 2)
            .reshape(HD, 16384)).astype(bf16)
        wk_c = wk[HD * c:HD * (c + 1)][perm]
        # wkp[p, k*128 + m] = wk_c[m, k*128+p]
        wkp = np.ascontiguousarray(
            wk_c.reshape(HD, KTILES, HD).transpose(2, 1, 0)
            .reshape(HD, 4096)).astype(bf16)
        wv_c = wv[HD * c:HD * (c + 1)]
        wvp = np.ascontiguousarray(
            wv_c.reshape(HD, KTILES, HD).transpose(2, 1, 0)
            .reshape(HD, 4096)).astype(bf16)
        woT_c = wo[:, qrows].T            # (512 ctx-feat, 4096 out-feat)
        # wop[p, of*512 + cf*128 + m] = woT_c[cf*128+p, of*128+m]
        wop = np.ascontiguousarray(
            woT_c.reshape(4, HD, KTILES, HD).transpose(1, 2, 0, 3)
            .reshape(HD, 16384)).astype(bf16)
        in_maps.append({
            "xT": xTb, "wqp": wqp, "wkp": wkp, "wvp": wvp, "wop": wop,
            "cosd": cosd, "sind": sind, "maskd": maskd, "onesc": onescol,
        })
    return in_maps


LAST_RESULTS = None


def kernel(x, wq, wk, wv, wo, attn_mask):
    global LAST_RESULTS
    from concourse import bass_utils
    if "nc" not in _cache:
        _cache["nc"] = _build()
    nc = _cache["nc"]
    in_maps = _host_inputs(np.asarray(x, np.float32), np.asarray(wq, np.float32),
                           np.asarray(wk, np.float32), np.asarray(wv, np.float32),
                           np.asarray(wo, np.float32))
    res = bass_utils.run_bass_kernel_spmd(
        nc, in_maps, core_ids=list(range(8)),
        trace=bool(os.environ.get("BASS_TRACE")))
    LAST_RESULTS = res
    acc = res.results[0]["outT"].astype(np.float64)
    for c in range(1, 8):
        acc = acc + res.results[c]["outT"].astype(np.float64)
    return np.ascontiguousarray(acc.T).astype(np.float32).reshape(1, S, D)
